# revision 2
# baseline (speedup 1.0000x reference)
"""Batched triu-scatter kernel for Trainium2.

x: [64, 2098176] f32 (packed upper-triangular rows of a 2048x2048 matrix)
-> out: [64, 2048, 2048] f32 with x scattered into the upper triangle,
zeros below the diagonal.

Distribution: row-interleaved across the 8 NeuronCores — core k handles
matrix rows r = k + 8*i (i = 0..255) of ALL 64 samples. This makes the
per-DMA batch dimension 64 (vs 8 for sample sharding), which matters
because the DMA engines assign descriptors to the 16 SDMA lanes by the
outermost access-pattern index: a 64-wide outer dim engages all 16
engines, an 8-wide one only half of them.

Host-side packing gives every core an IDENTICAL program (required for
SPMD): slot i is padded to S_i = 2048 - 8*i = L + k elements (k zeros up
front), so per-core access patterns don't depend on k. Layouts are
slot-major with the 64 samples contiguous inside each slot — keeping
each instruction's 64 descriptors within ~512KB of address space, which
the DMA engines need for full rate (descriptors strided MBs apart run
3x slower).

The kernel writes each output row right-aligned at its true columns;
the k pad zeros land on legitimately-zero cells left of the diagonal,
and everything further left is never written: run_bass_kernel_spmd
pre-zeroes (and donates) ExternalOutput buffers, so untouched cells
read back as zero. Net HBM traffic per core: 67MB read + 67MB written
(the 67MB of below-diagonal zeros are never transferred).
"""

import numpy as np

import concourse.bass as bass
import concourse.mybir as mybir
from concourse.bass_utils import run_bass_kernel_spmd

M = 2048
NT = M * (M + 1) // 2  # 2098176
B = 64
N_CORES = 8
NSLOTS = M // N_CORES  # 256
S = [M - 8 * i for i in range(NSLOTS)]  # slot widths (same for all cores)
SLOT_OFF = np.concatenate([[0], np.cumsum([64 * s for s in S])])  # elem offsets
N_IN = int(SLOT_OFF[-1])  # 64 * 263168 elements per core
ROW_OFF = [r * M - r * (r - 1) // 2 for r in range(M)]  # packed triu row offsets

_nc_cache = None


def _build():
    nc = bass.Bass()
    x = nc.dram_tensor("x", [N_IN], mybir.dt.float32, kind="ExternalInput")
    y = nc.dram_tensor("y", [NSLOTS, B, M], mybir.dt.float32, kind="ExternalOutput")
    with nc.semaphore("sem_a") as sem_a, nc.semaphore("sem_b") as sem_b:
        counts = {0: 0, 1: 0}
        sems = {0: sem_a, 1: sem_b}
        engs = {0: nc.sync, 1: nc.scalar}
        for i in range(NSLOTS):
            ring = i % 2
            w = S[i]
            src = bass.AP(x[:].tensor, int(SLOT_OFF[i]), [[w, B], [1, w]])
            dst = bass.AP(y[:, :, :].tensor, i * B * M + (M - w), [[M, B], [1, w]])
            engs[ring].dma_start(dst, src).then_inc(sems[ring], 16)
            counts[ring] += 1
        nc.sync.wait_ge(sem_a, 16 * counts[0])
        nc.scalar.wait_ge(sem_b, 16 * counts[1])
    return nc


def _get_nc():
    global _nc_cache
    if _nc_cache is None:
        _nc_cache = _build()
    return _nc_cache


def _pack_core(x, k):
    """Pack core k's input: slot i holds [64, S_i] = [k zeros || row k+8i]."""
    xk = np.zeros((N_IN,), np.float32)
    for i in range(NSLOTS):
        r = k + 8 * i
        L = M - r
        seg = xk[SLOT_OFF[i] : SLOT_OFF[i + 1]].reshape(B, S[i])
        o = ROW_OFF[r]
        seg[:, k:] = x[:, o : o + L]
    return xk


def kernel(x: np.ndarray, _trace: bool = False):
    assert x.shape == (B, NT), x.shape
    x = np.ascontiguousarray(x, dtype=np.float32)
    nc = _get_nc()
    in_maps = [{"x": _pack_core(x, k)} for k in range(N_CORES)]
    res = run_bass_kernel_spmd(
        nc, in_maps, core_ids=list(range(N_CORES)), trace=_trace
    )
    out = np.empty((B, NSLOTS, N_CORES, M), np.float32)
    for k in range(N_CORES):
        # y_k is [slot, sample, col] -> out[sample, slot, k, col]
        out[:, :, k, :] = res.results[k]["y"].transpose(1, 0, 2)
    out = out.reshape(B, M, M)
    if _trace:
        return out, res
    return out


# revision 3
# speedup vs baseline: 1.0225x; 1.0225x over previous
"""Batched triu-scatter kernel for Trainium2.

x: [64, 2098176] f32 (packed upper-triangular rows of a 2048x2048 matrix)
-> out: [64, 2048, 2048] f32 with x scattered into the upper triangle,
zeros below the diagonal.

Distribution: row-interleaved across the 8 NeuronCores — core k handles
matrix rows r = k + 8*i (i = 0..255) of ALL 64 samples. This makes the
per-DMA batch dimension 64 (vs 8 for sample sharding), which matters
because the DMA engines assign descriptors to the 16 SDMA lanes by the
outermost access-pattern index: a 64-wide outer dim engages all 16
engines, an 8-wide one only half of them.

Host-side packing gives every core an IDENTICAL program (required for
SPMD): slot i is padded to S_i = 2048 - 8*i = L + k elements (k zeros up
front), so per-core access patterns don't depend on k. Layouts are
slot-major with the 64 samples contiguous inside each slot — keeping
each instruction's 64 descriptors within ~512KB of address space, which
the DMA engines need for full rate (descriptors strided MBs apart run
3x slower).

The kernel writes each output row right-aligned at its true columns;
the k pad zeros land on legitimately-zero cells left of the diagonal,
and everything further left is never written: run_bass_kernel_spmd
pre-zeroes (and donates) ExternalOutput buffers, so untouched cells
read back as zero. Net HBM traffic per core: 67MB read + 67MB written
(the 67MB of below-diagonal zeros are never transferred).
"""

import numpy as np

import concourse.bass as bass
import concourse.mybir as mybir
from concourse.bass_utils import run_bass_kernel_spmd

M = 2048
NT = M * (M + 1) // 2  # 2098176
B = 64
N_CORES = 8
NSLOTS = M // N_CORES  # 256
S = [M - 8 * i for i in range(NSLOTS)]  # slot widths (same for all cores)
SLOT_OFF = np.concatenate([[0], np.cumsum([64 * s for s in S])])  # elem offsets
N_IN = int(SLOT_OFF[-1])  # 64 * 263168 elements per core
ROW_OFF = [r * M - r * (r - 1) // 2 for r in range(M)]  # packed triu row offsets

_nc_cache = None


def _build():
    nc = bass.Bass()
    x = nc.dram_tensor("x", [N_IN], mybir.dt.float32, kind="ExternalInput")
    y = nc.dram_tensor("y", [NSLOTS, B, M], mybir.dt.float32, kind="ExternalOutput")
    with nc.semaphore("sem_a") as sem_a, nc.semaphore("sem_b") as sem_b:
        counts = {0: 0, 1: 0}
        sems = {0: sem_a, 1: sem_b}
        engs = {0: nc.sync, 1: nc.scalar}
        for i in range(NSLOTS):
            ring = i % 2
            w = S[i]
            src = bass.AP(x[:].tensor, int(SLOT_OFF[i]), [[w, B], [1, w]])
            dst = bass.AP(y[:, :, :].tensor, i * B * M + (M - w), [[M, B], [1, w]])
            engs[ring].dma_start(dst, src).then_inc(sems[ring], 16)
            counts[ring] += 1
        nc.sync.wait_ge(sem_a, 16 * counts[0])
        nc.scalar.wait_ge(sem_b, 16 * counts[1])
    return nc


def _get_nc():
    global _nc_cache
    if _nc_cache is None:
        _nc_cache = _build()
    return _nc_cache


def _pack_core(x, k):
    """Pack core k's input: slot i holds [64, S_i] = [k zeros || row k+8i]."""
    xk = np.zeros((N_IN,), np.float32)
    for i in range(NSLOTS):
        r = k + 8 * i
        L = M - r
        seg = xk[SLOT_OFF[i] : SLOT_OFF[i + 1]].reshape(B, S[i])
        o = ROW_OFF[r]
        seg[:, k:] = x[:, o : o + L]
    return xk


def kernel(x: np.ndarray, _trace: bool = False):
    assert x.shape == (B, NT), x.shape
    x = np.ascontiguousarray(x, dtype=np.float32)
    nc = _get_nc()
    in_maps = [{"x": _pack_core(x, k)} for k in range(N_CORES)]
    # The first execution after an unclean device state occasionally fails
    # with NRT_EXEC_UNIT_UNRECOVERABLE; a retry on a re-initialized device
    # succeeds, so try up to 3 times.
    last_exc = None
    for _attempt in range(3):
        try:
            res = run_bass_kernel_spmd(
                nc, in_maps, core_ids=list(range(N_CORES)), trace=_trace
            )
            break
        except Exception as e:  # noqa: BLE001
            last_exc = e
    else:
        raise last_exc
    out = np.empty((B, NSLOTS, N_CORES, M), np.float32)
    for k in range(N_CORES):
        # y_k is [slot, sample, col] -> out[sample, slot, k, col]
        out[:, :, k, :] = res.results[k]["y"].transpose(1, 0, 2)
    out = out.reshape(B, M, M)
    if _trace:
        return out, res
    return out
